# revision 11
# baseline (speedup 1.0000x reference)
"""Cross-attention Trainium2 kernel (Bass/Tile), SPMD over 8 NeuronCores.

Problem: b=8, i=j=2048, query/context dim 512, inner dim 256.
Sharding: data-parallel over batch - one batch element per core, no
collectives. Each core computes, for its batch element:

    q = x @ Wq ; k = ctx @ Wk ; v = ctx @ Wv
    sim = (q @ k^T) * d^-0.5 ; attn = softmax_j(sim) masked on j
    out = attn @ v ; y = out @ Wo + bo + x

fp8 dataflow (all big matmuls fp8e4m3 DoubleRow: K=256 contracted per
instruction at 0.5 cyc/row):
  1. Weights staged f32 -> fp8 scaled by 32 (keeps N(0,0.02^2) weights
     out of the fp8 denormal range).
  2. x/ctx cast f32 -> fp8 on GpSimd (the context cast folds the mask
     in as a per-j row scale, so masked j contribute exactly zero to
     PV); PE-transposed to k-major, 16 [128,128] tiles packed per PSUM
     bank (start=True only on the bank's first write; later writes
     land on pending-zero bytes), one u32-bitcast eviction per group.
  3. qT/kT (d-major) and v (j-major) projections via DoubleRow pairs;
     PSUM->SBUF evictions cast to fp8 for free.
  4. simT[j,i] per (i-block, jt-pair) -> exp: ScalarE Act.Exp with the
     combined scale (d^-0.5/32^2) writing fp8 directly, a share of
     tiles on DVE via the Schraudolph bit trick (round(x*8/ln2+B) as
     int8 IS fp8e4m3 of e^x to ~5%); denominator = maskT @ attn
     DoubleRow rank-1; both consume the same quantized attn the PV
     matmuls use.
  5. outT accumulated over j in PSUM, evicted /64 to fp8; y per i-tile
     = oT^T @ wo (DoubleRow) + rank-1 den (x) 16*bo (so the bias lands
     pre-normalization exactly); DVE fuses *1/(16 den) + x residual.
DMA: inputs on the SP HWDGE queue ordered [wk wv wq mask, x0, ctx0-3,
x1, wo bo, x2 x3] so attention block 0 streams behind the ctx DMAs;
y writebacks ride the Activation HWDGE queue so they never block
input loads (all transfers serialize on the DMA engines anyway).
"""

import sys

import numpy as np

if "/opt/trn_rl_repo" not in sys.path:
    sys.path.insert(0, "/opt/trn_rl_repo")

_P = 128          # partitions
_B = 8            # batch == number of cores
_I = 2048         # query sequence length
_J = 2048         # context sequence length
_K = 512          # query/context feature dim
_D = 256          # inner dim
_NB = 512         # i-block size
_KT = _K // _P    # 4 contraction tiles for projections
_DT = _D // _P    # 2 inner-dim tiles
_IT = _I // _P    # 16 i tiles
_JT = _J // _P    # 16 j tiles
_WS = 32.0        # weight prescale into fp8 range
# exp argument: sim_psum * d^-0.5 / WS^2
_SEXP = float(_D) ** -0.5 / (_WS * _WS)
# Schraudolph fp8e4m3 exp: bits = round(x * 8/ln2 + 55.63)
_SCH_A = 8.0 / float(np.log(2.0))
_SCH_B = 55.63

_CACHE = {}


def _split_multi_waits(nc, limits):
    """Walrus in this container rejects instructions carrying more sem
    waits than its per-template slot count (e.g. Drain allows 1). Move
    excess waits onto wait-only Drain carriers on the same engine,
    inserted just before the instruction - semantically identical."""
    from concourse import mybir

    n_split = 0
    for func in nc.m.functions:
        for block in func.blocks:
            out = []
            for inst in block.instructions:
                si = inst.sync_info
                maxw = limits.get(type(inst).__name__, limits.get("*"))
                if (
                    maxw is not None
                    and si is not None
                    and si.on_wait
                    and len(si.on_wait) > maxw
                ):
                    waits = list(si.on_wait)
                    keep, rest = waits[:maxw], waits[maxw:]
                    for i in range(0, len(rest), 1):
                        car = mybir.InstDrain(
                            name=f"I-waitcar-{nc.next_id()}", ins=[], outs=[]
                        )
                        car.engine = inst.engine
                        car.sync_info = mybir.SyncInfo(
                            on_wait=[rest[i]], on_update=[]
                        )
                        nc.register_instruction(car)
                        out.append(car)
                        n_split += 1
                    inst.sync_info = mybir.SyncInfo(
                        on_wait=keep, on_update=list(si.on_update or [])
                    )
                out.append(inst)
            block.instructions = out
    return n_split


def _build_nc():
    import concourse.bass as bass
    import concourse.tile as tile
    from concourse import mybir
    from concourse.masks import make_identity

    dt = mybir.dt
    Alu = mybir.AluOpType
    Act = mybir.ActivationFunctionType
    DR = mybir.MatmulPerfMode.DoubleRow
    F8 = dt.float8e4

    nc = bass.Bass("TRN2", target_bir_lowering=False)

    x_d = nc.dram_tensor("x", [_I, _K], dt.float32, kind="ExternalInput")
    c_d = nc.dram_tensor("context", [_J, _K], dt.float32, kind="ExternalInput")
    m_d = nc.dram_tensor("mask", [_J], dt.uint8, kind="ExternalInput")
    wq_d = nc.dram_tensor("Wq", [_K, _D], dt.float32, kind="ExternalInput")
    wk_d = nc.dram_tensor("Wk", [_K, _D], dt.float32, kind="ExternalInput")
    wv_d = nc.dram_tensor("Wv", [_K, _D], dt.float32, kind="ExternalInput")
    wo_d = nc.dram_tensor("Wo", [_D, _K], dt.float32, kind="ExternalInput")
    bo_d = nc.dram_tensor("bo", [_K], dt.float32, kind="ExternalInput")
    y_d = nc.dram_tensor("out", [_I, _K], dt.float32, kind="ExternalOutput")

    with tile.TileContext(nc) as tc:
        with (
            tc.tile_pool(name="persist", bufs=1) as persist,
            tc.tile_pool(name="cstg", bufs=2) as cstg,
            tc.tile_pool(name="stg8", bufs=2) as stg8,
            tc.tile_pool(name="wstage", bufs=2) as wstage,
            tc.tile_pool(name="attnT", bufs=2) as attnp,
            tc.tile_pool(name="otp", bufs=2) as otp,
            tc.tile_pool(name="yout", bufs=2) as youtp,
            tc.tile_pool(name="small", bufs=4) as small,
            tc.tile_pool(name="pkp", bufs=1, space="PSUM") as pkp,
            tc.tile_pool(name="smp", bufs=2, space="PSUM") as smp,
            tc.tile_pool(name="accp", bufs=2, space="PSUM") as accp,
            tc.tile_pool(name="denp", bufs=1, space="PSUM") as denp,
        ):
            # ---------------- constants ----------------
            ident8 = persist.tile([_P, _P], F8, tag="ident8")
            make_identity(nc, ident8)
            one16 = persist.tile([1, 1], dt.bfloat16, tag="one16")
            nc.vector.memset(one16, 16.0)

            # ---------------- persistent tensors ----------------
            x_nat = persist.tile([_P, _IT, _K], dt.float32, tag="xnat")
            # transposed activations: fp8 data at even bytes (the PE's
            # fp8-transpose writes element step 2), stored as u16 with
            # junk high bytes; consumers use stride-2 fp8 APs.
            xTs = persist.tile([_P, _KT, _I], dt.uint16, tag="xTs")
            cTs = persist.tile([_P, _KT, _J], dt.uint16, tag="cTs")
            xT8 = xTs.bitcast(F8)
            cT8 = cTs.bitcast(F8)

            def pair_ap(t8, kt0, j0, nj):
                """[K=128, 2, nj] fp8 AP over a stride-2 transposed tensor:
                pair dim = (kt0, kt0+1), free dim = nj j's from j0."""
                full = t8[:]
                return bass.AP(
                    tensor=full.tensor,
                    offset=full.offset + kt0 * 2 * _J + 2 * j0,
                    ap=[full.ap[0], [2 * _J, 2], [2, nj]],
                )
            qT = persist.tile([_P, _DT, _I], F8, tag="qT")
            kT = persist.tile([_P, _DT, _J], F8, tag="kT")
            v = persist.tile([_P, _JT, _D], F8, tag="v")
            recips = persist.tile([_P, _IT], dt.float32, tag="recips")

            wq8 = persist.tile([_P, _KT, _D], F8, tag="wq8")
            wk8 = persist.tile([_P, _KT, _D], F8, tag="wk8")
            wv8 = persist.tile([_P, _KT, _D], F8, tag="wv8")
            wo8 = persist.tile([_P, _DT, _K], F8, tag="wo8")
            mask01 = persist.tile([_P, _JT], dt.float32, tag="mask01")
            # mask broadcast along a 128-wide M dim: the denominator runs
            # as an M=128 DoubleRow matmul (M=1 fp8 ldweights is rejected
            # by the ISA's dual-fp8 restrictions); every PSUM row gets the
            # same masked column sum.
            mask8b = persist.tile([_P, _JT, _P], F8, tag="mask8b")
            bo16 = persist.tile([1, _K], dt.bfloat16, tag="bo16")

            # the denominator bank: pden ([1,512] accum) and the 4
            # K=1 transposed-denominator columns share one PSUM bank;
            # every reuse is covered by the pending-zero semantics of
            # start=True plus the strict RAW/WAR ordering on the tile.
            denbank = denp.tile([_P, _NB], dt.float32, tag="denbank")

            # ---------------- weight / mask staging ----------------
            def load_w_early():
                for w_dram, w_sb in ((wk_d, wk8), (wv_d, wv8), (wq_d, wq8)):
                    ws = wstage.tile([_P, _KT, _D], dt.float32, tag="ws",
                                     bufs=3, name=f"ws_{w_sb.name}")
                    nc.sync.dma_start(
                        out=ws, in_=w_dram[:].rearrange("(t p) d -> p t d", p=_P)
                    )
                    if w_sb is wq8:
                        nc.vector.tensor_scalar(
                            out=w_sb, in0=ws, scalar1=_WS, scalar2=None,
                            op0=Alu.mult,
                        )
                    else:
                        nc.scalar.activation(
                            out=w_sb, in_=ws, func=Act.Copy, scale=_WS
                        )
                msk8 = small.tile([_P, _JT], dt.uint8, tag="msk8")
                nc.sync.dma_start(
                    out=msk8, in_=m_d[:].rearrange("(t p) -> p t", p=_P)
                )
                nc.vector.tensor_copy(out=mask01, in_=msk8)
                m01 = mask01[:]
                nc.vector.tensor_copy(
                    out=mask8b,
                    in_=bass.AP(tensor=m01.tensor, offset=m01.offset,
                                ap=[m01.ap[0], [1, _JT], [0, _P]]),
                )

            def load_w_late():
                ws = wstage.tile([_P, _DT, _K], dt.float32, tag="wso")
                nc.sync.dma_start(
                    out=ws, in_=wo_d[:].rearrange("(t p) k -> p t k", p=_P)
                )
                nc.vector.tensor_scalar(
                    out=wo8, in0=ws, scalar1=_WS, scalar2=None, op0=Alu.mult
                )
                bs = small.tile([1, _K], dt.float32, tag="bs")
                nc.sync.dma_start(out=bs, in_=bo_d[:].rearrange("(o k) -> o k", o=1))
                # rank-1 bias term is den (x) 16*bo; *1/(16 den) later
                nc.vector.tensor_scalar(
                    out=bo16, in0=bs, scalar1=16.0, scalar2=None, op0=Alu.mult
                )

            # ---------------- front-end helpers ----------------
            tr_evict_n = [0]

            def transpose_half(src8, rt0, dstTs, col0):
                """8 step-2 fp8 PE transposes (2 row-tiles x 4 kt) packed
                into one PSUM bank; one u16 eviction (2x DVE mode) into
                dstTs[:, :, col0:col0+256]."""
                pk = pkp.tile([_P, _NB], dt.float32, tag="pk")
                pk8 = pk.bitcast(F8)
                for kt in range(_KT):
                    for rt in range(2):
                        slot = kt * 2 + rt
                        sl = pk8[:, slot * 256:(slot + 1) * 256]
                        stepped = bass.AP(
                            tensor=sl.tensor, offset=sl.offset,
                            ap=[sl.ap[0], [2, _P]],
                        )
                        nc.tensor.matmul(
                            stepped,
                            lhsT=src8[:, rt0 + rt, kt * _P:(kt + 1) * _P],
                            rhs=ident8,
                            is_transpose=True,
                            start=(slot == 0), stop=True,
                            skip_group_check=True,
                        )
                # src slots are [kt][rt][j] contiguous = [P, 4, 256] u16;
                # dst is [P, 4(kt), 256] u16 with kt stride 2048.
                dst = dstTs[:, :, col0:col0 + 2 * _P]
                pku = pk.bitcast(dt.uint16)[:]
                src = bass.AP(
                    tensor=pku.tensor, offset=pku.offset,
                    ap=[pku.ap[0], [256, _KT], [1, 256]],
                )
                if tr_evict_n[0] % 2 == 0:
                    nc.vector.tensor_copy(out=dst, in_=src)
                else:
                    nc.scalar.copy(out=dst, in_=src)
                tr_evict_n[0] += 1

            def ctx_group(g):
                cst = cstg.tile([_P, 4, _K], dt.float32, tag="cst")
                if g == 0:
                    splits = ((0, 1), (1, 1), (2, 2))
                else:
                    splits = ((0, 2), (2, 2))
                for o, n in splits:
                    r0 = (g * 4 + o) * _P
                    nc.sync.dma_start(
                        out=cst[:, o:o + n, :],
                        in_=c_d[r0:r0 + n * _P, :].rearrange(
                            "(t p) k -> p t k", p=_P
                        ),
                    )
                c8g = stg8.tile([_P, 4, _K], F8, tag="c8g")
                for rt in range(4):
                    jt = g * 4 + rt
                    # mask folded in: masked j rows of ctx become 0 so
                    # both kT columns and v rows vanish for them.
                    nc.gpsimd.tensor_scalar(
                        out=c8g[:, rt, :], in0=cst[:, rt, :],
                        scalar1=mask01[:, jt:jt + 1], scalar2=None,
                        op0=Alu.mult,
                    )
                transpose_half(c8g, 0, cTs, g * 4 * _P)
                transpose_half(c8g, 2, cTs, g * 4 * _P + 2 * _P)
                # kT projection for this 512-j block
                pj = smp.tile([_P, 2 * _NB], dt.float32, tag="sm")
                for dh in range(_DT):
                    for t in range(2):
                        nc.tensor.matmul(
                            pj[:, dh * _NB:(dh + 1) * _NB],
                            lhsT=wk8[:, 2 * t:2 * t + 2, dh * _P:(dh + 1) * _P],
                            rhs=pair_ap(cT8, 2 * t, g * _NB, _NB),
                            perf_mode=DR, start=(t == 0), stop=(t == 1),
                            skip_group_check=True,
                        )
                nc.scalar.copy(
                    out=kT[:, :, g * _NB:(g + 1) * _NB],
                    in_=pj[:].rearrange("p (d n) -> p d n", d=2),
                )
                # v projection, two j-tiles packed per PSUM bank
                for jp in range(2):
                    jt0 = g * 4 + 2 * jp
                    pv = smp.tile([_P, _NB], dt.float32, tag="sm",
                                  name=f"pv{g}_{jp}")
                    for jo in range(2):
                        for t in range(2):
                            nc.tensor.matmul(
                                pv[:, jo * _D:(jo + 1) * _D],
                                lhsT=pair_ap(cT8, 2 * t, (jt0 + jo) * _P, _P),
                                rhs=wv8[:, 2 * t:2 * t + 2, :],
                                perf_mode=DR,
                                start=(jo == 0 and t == 0), stop=(t == 1),
                                skip_group_check=True,
                            )
                    nc.vector.tensor_copy(
                        out=v[:, jt0:jt0 + 2, :],
                        in_=pv[:].rearrange("p (j d) -> p j d", j=2),
                    )

            def x_group(b, dma_only=False):
                r0 = b * 4 * _P
                for h in range(2):
                    nc.sync.dma_start(
                        out=x_nat[:, b * 4 + 2 * h:b * 4 + 2 * h + 2, :],
                        in_=x_d[r0 + 2 * h * _P:r0 + (2 * h + 2) * _P, :]
                        .rearrange("(t p) k -> p t k", p=_P),
                    )

            def x_front(b):
                x8g = stg8.tile([_P, 4, _K], F8, tag="x8g")
                nc.gpsimd.tensor_copy(out=x8g, in_=x_nat[:, b * 4:b * 4 + 4, :])
                transpose_half(x8g, 0, xTs, b * 4 * _P)
                transpose_half(x8g, 2, xTs, b * 4 * _P + 2 * _P)
                pj = smp.tile([_P, 2 * _NB], dt.float32, tag="sm",
                              name=f"pq{b}")
                for dh in range(_DT):
                    for t in range(2):
                        nc.tensor.matmul(
                            pj[:, dh * _NB:(dh + 1) * _NB],
                            lhsT=wq8[:, 2 * t:2 * t + 2, dh * _P:(dh + 1) * _P],
                            rhs=pair_ap(xT8, 2 * t, b * _NB, _NB),
                            perf_mode=DR, start=(t == 0), stop=(t == 1),
                            skip_group_check=True,
                        )
                nc.scalar.copy(
                    out=qT[:, :, b * _NB:(b + 1) * _NB],
                    in_=pj[:].rearrange("p (d n) -> p d n", d=2),
                )

            # ---------------- attention ----------------
            aTs = {}
            accs = {}

            def start_block(b):
                aTs[b] = attnp.tile([_P, _JT, _NB], F8, tag="aT",
                                    name=f"aT{b}")
                accs[b] = [
                    accp.tile([_P, _NB], dt.float32, tag="acc",
                              name=f"acc{b}_{dh}")
                    for dh in range(_DT)
                ]

            def sim_exp(b, pr):
                sm = smp.tile([_P, 2 * _NB], dt.float32, tag="sm",
                              name=f"sim{b}_{pr}")
                for jo in range(2):
                    jt = 2 * pr + jo
                    nc.tensor.matmul(
                        sm[:, jo * _NB:(jo + 1) * _NB],
                        lhsT=kT[:, :, jt * _P:(jt + 1) * _P],
                        rhs=qT[:, :, b * _NB:(b + 1) * _NB],
                        perf_mode=DR, start=True, stop=True,
                        skip_group_check=True,
                    )
                dst = aTs[b][:, 2 * pr:2 * pr + 2, :]
                if (b * 8 + pr) % 3 == 2:
                    # Schraudolph fp8 exp on DVE: int8 bits of e4m3
                    nc.vector.tensor_scalar(
                        out=dst.bitcast(dt.int8), in0=sm,
                        scalar1=_SEXP * _SCH_A, scalar2=_SCH_B,
                        op0=Alu.mult, op1=Alu.add,
                    )
                else:
                    nc.scalar.activation(
                        out=dst, in_=sm, func=Act.Exp, bias=0.0, scale=_SEXP
                    )

            def pv_den(b, pr):
                aT = aTs[b]
                for dh in range(_DT):
                    nc.tensor.matmul(
                        accs[b][dh],
                        lhsT=v[:, 2 * pr:2 * pr + 2, dh * _P:(dh + 1) * _P],
                        rhs=aT[:, 2 * pr:2 * pr + 2, :],
                        perf_mode=DR, start=(pr == 0), stop=(pr == 7),
                        skip_group_check=True,
                    )
                nc.tensor.matmul(
                    denbank[:, :],
                    lhsT=mask8b[:, 2 * pr:2 * pr + 2, :],
                    rhs=aT[:, 2 * pr:2 * pr + 2, :],
                    perf_mode=DR, start=(pr == 0), stop=(pr == 7),
                    skip_group_check=True,
                )

            def finish_block(b):
                # oT = accs/64 in fp8 (ScalarE), frees the acc banks
                oTt = otp.tile([_P, _DT, _NB], F8, tag="oT", name=f"oT{b}")
                for dh in range(_DT):
                    nc.scalar.activation(
                        out=oTt[:, dh, :], in_=accs[b][dh],
                        func=Act.Copy, scale=1.0 / 64.0,
                    )
                del accs[b], aTs[b]
                den16 = small.tile([1, _NB], dt.bfloat16, tag="den16",
                                   name=f"den16_{b}")
                nc.vector.tensor_copy(out=den16, in_=denbank[:1, :])
                # transpose den to i-major via K=1 matmuls: trd = 16*den
                for tt in range(4):
                    nc.tensor.matmul(
                        denbank[:, tt:tt + 1],
                        lhsT=den16[:, tt * _P:(tt + 1) * _P],
                        rhs=one16,
                        start=(tt == 0), stop=True,
                        skip_group_check=True,
                    )
                nc.vector.reciprocal(
                    out=recips[:, 4 * b:4 * b + 4], in_=denbank[:, 0:4]
                )
                ys = youtp.tile([_P, 4, _K], dt.float32, tag="ys",
                                name=f"ys{b}")
                for tt in range(4):
                    t = 4 * b + tt
                    yp = smp.tile([_P, _K], dt.float32, tag="sm",
                                  name=f"yp{t}")
                    nc.tensor.matmul(
                        yp,
                        lhsT=oTt[:, :, tt * _P:(tt + 1) * _P],
                        rhs=wo8,
                        perf_mode=DR, start=True, stop=False,
                        skip_group_check=True,
                    )
                    nc.tensor.matmul(
                        yp,
                        lhsT=den16[:, tt * _P:(tt + 1) * _P],
                        rhs=bo16,
                        start=False, stop=True,
                        skip_group_check=True,
                    )
                    nc.vector.scalar_tensor_tensor(
                        out=ys[:, tt, :], in0=yp, scalar=recips[:, t:t + 1],
                        in1=x_nat[:, t, :], op0=Alu.mult, op1=Alu.add,
                    )
                nc.scalar.dma_start(
                    out=y_d[b * 4 * _P:(b + 1) * 4 * _P, :].rearrange(
                        "(t p) k -> p t k", p=_P
                    ),
                    in_=ys,
                )

            # ---------------- schedule ----------------
            load_w_early()
            x_group(0)
            x_front(0)
            start_block(0)
            for g in range(4):
                ctx_group(g)
                for pr in (2 * g, 2 * g + 1):
                    sim_exp(0, pr)
                    if pr > 0:
                        pv_den(0, pr - 1)
            x_group(1)
            load_w_late()
            x_group(2)
            x_group(3)
            pv_den(0, 7)
            finish_block(0)
            for b in range(1, 4):
                x_front(b)
                start_block(b)
                for pr in range(8):
                    sim_exp(b, pr)
                    if pr > 0:
                        pv_den(b, pr - 1)
                pv_den(b, 7)
                finish_block(b)

    _split_multi_waits(nc, {"*": 1})
    nc.finalize()
    return nc


def kernel(x, context, mask, Wq, Wk, Wv, Wo, bo):
    from concourse.bass_utils import run_bass_kernel_spmd

    if "nc" not in _CACHE:
        _CACHE["nc"] = _build_nc()
    nc = _CACHE["nc"]

    x = np.ascontiguousarray(np.asarray(x, dtype=np.float32))
    context = np.ascontiguousarray(np.asarray(context, dtype=np.float32))
    mask_u8 = np.ascontiguousarray(np.asarray(mask).astype(np.uint8))
    shared = {
        "Wq": np.ascontiguousarray(np.asarray(Wq, dtype=np.float32)),
        "Wk": np.ascontiguousarray(np.asarray(Wk, dtype=np.float32)),
        "Wv": np.ascontiguousarray(np.asarray(Wv, dtype=np.float32)),
        "Wo": np.ascontiguousarray(np.asarray(Wo, dtype=np.float32)),
        "bo": np.ascontiguousarray(np.asarray(bo, dtype=np.float32)),
    }
    in_maps = [
        {"x": x[b], "context": context[b], "mask": mask_u8[b], **shared}
        for b in range(_B)
    ]
    res = run_bass_kernel_spmd(nc, in_maps, core_ids=list(range(_B)))
    return np.stack([res.results[b]["out"] for b in range(_B)], axis=0)


# revision 19
# speedup vs baseline: 1.0160x; 1.0160x over previous
"""Cross-attention Trainium2 kernel (Bass/Tile), SPMD over 8 NeuronCores.

Problem: b=8, i=j=2048, query/context dim 512, inner dim 256.
Sharding: data-parallel over batch - one batch element per core, no
collectives. Each core computes, for its batch element:

    q = x @ Wq ; k = ctx @ Wk ; v = ctx @ Wv
    sim = (q @ k^T) * d^-0.5 ; attn = softmax_j(sim) masked on j
    out = attn @ v ; y = out @ Wo + bo + x

fp8 dataflow (all big matmuls fp8e4m3 DoubleRow: K=256 contracted per
instruction at 0.5 cyc/row):
  1. Weights staged f32 -> fp8 scaled by 32 (keeps N(0,0.02^2) weights
     out of the fp8 denormal range).
  2. x/ctx cast f32 -> fp8 on GpSimd (the context cast folds the mask
     in as a per-j row scale, so masked j contribute exactly zero to
     PV); PE-transposed to k-major, 16 [128,128] tiles packed per PSUM
     bank (start=True only on the bank's first write; later writes
     land on pending-zero bytes), one u32-bitcast eviction per group.
  3. qT/kT (d-major) and v (j-major) projections via DoubleRow pairs;
     PSUM->SBUF evictions cast to fp8 for free.
  4. simT[j,i] per (i-block, jt-pair) -> exp: ScalarE Act.Exp with the
     combined scale (d^-0.5/32^2) writing fp8 directly, a share of
     tiles on DVE via the Schraudolph bit trick (round(x*8/ln2+B) as
     int8 IS fp8e4m3 of e^x to ~5%); denominator = maskT @ attn
     DoubleRow rank-1; both consume the same quantized attn the PV
     matmuls use.
  5. outT accumulated over j in PSUM, evicted /64 to fp8; y per i-tile
     = oT^T @ wo (DoubleRow) + rank-1 den (x) 16*bo (so the bias lands
     pre-normalization exactly); DVE fuses *1/(16 den) + x residual.
DMA: inputs on the SP HWDGE queue ordered [wk wv wq mask, x0, ctx0-3,
x1, wo bo, x2 x3] so attention block 0 streams behind the ctx DMAs;
y writebacks ride the Activation HWDGE queue so they never block
input loads (all transfers serialize on the DMA engines anyway).
"""

import sys

import numpy as np

if "/opt/trn_rl_repo" not in sys.path:
    sys.path.insert(0, "/opt/trn_rl_repo")

_P = 128          # partitions
_B = 8            # batch == number of cores
_I = 2048         # query sequence length
_J = 2048         # context sequence length
_K = 512          # query/context feature dim
_D = 256          # inner dim
_NB = 512         # i-block size
_KT = _K // _P    # 4 contraction tiles for projections
_DT = _D // _P    # 2 inner-dim tiles
_IT = _I // _P    # 16 i tiles
_JT = _J // _P    # 16 j tiles
_WS = 32.0        # weight prescale into fp8 range
# exp argument: sim_psum * d^-0.5 / WS^2
_SEXP = float(_D) ** -0.5 / (_WS * _WS)
# Schraudolph fp8e4m3 exp: bits = round(x * 8/ln2 + 55.63)
_SCH_A = 8.0 / float(np.log(2.0))
_SCH_B = 55.63

_CACHE = {}


def _split_multi_waits(nc, limits):
    """Walrus in this container rejects instructions carrying more sem
    waits than its per-template slot count (e.g. Drain allows 1). Move
    excess waits onto wait-only Drain carriers on the same engine,
    inserted just before the instruction - semantically identical."""
    from concourse import mybir

    n_split = 0
    for func in nc.m.functions:
        for block in func.blocks:
            out = []
            for inst in block.instructions:
                si = inst.sync_info
                maxw = limits.get(type(inst).__name__, limits.get("*"))
                if (
                    maxw is not None
                    and si is not None
                    and si.on_wait
                    and len(si.on_wait) > maxw
                ):
                    waits = list(si.on_wait)
                    keep, rest = waits[:maxw], waits[maxw:]
                    for i in range(0, len(rest), 1):
                        car = mybir.InstDrain(
                            name=f"I-waitcar-{nc.next_id()}", ins=[], outs=[]
                        )
                        car.engine = inst.engine
                        car.sync_info = mybir.SyncInfo(
                            on_wait=[rest[i]], on_update=[]
                        )
                        nc.register_instruction(car)
                        out.append(car)
                        n_split += 1
                    inst.sync_info = mybir.SyncInfo(
                        on_wait=keep, on_update=list(si.on_update or [])
                    )
                out.append(inst)
            block.instructions = out
    return n_split


def _build_nc():
    import concourse.bass as bass
    import concourse.tile as tile
    from concourse import mybir
    from concourse.masks import make_identity

    dt = mybir.dt
    Alu = mybir.AluOpType
    Act = mybir.ActivationFunctionType
    DR = mybir.MatmulPerfMode.DoubleRow
    F8 = dt.float8e4

    nc = bass.Bass("TRN2", target_bir_lowering=False)

    x_d = nc.dram_tensor("x", [_I, _K], dt.float32, kind="ExternalInput")
    c_d = nc.dram_tensor("context", [_J, _K], dt.float32, kind="ExternalInput")
    m_d = nc.dram_tensor("mask", [_J], dt.uint8, kind="ExternalInput")
    wq_d = nc.dram_tensor("Wq", [_K, _D], dt.float32, kind="ExternalInput")
    wk_d = nc.dram_tensor("Wk", [_K, _D], dt.float32, kind="ExternalInput")
    wv_d = nc.dram_tensor("Wv", [_K, _D], dt.float32, kind="ExternalInput")
    wo_d = nc.dram_tensor("Wo", [_D, _K], dt.float32, kind="ExternalInput")
    bo_d = nc.dram_tensor("bo", [_K], dt.float32, kind="ExternalInput")
    y_d = nc.dram_tensor("out", [_I, _K], dt.float32, kind="ExternalOutput")

    with tile.TileContext(nc) as tc:
        with (
            tc.tile_pool(name="persist", bufs=1) as persist,
            tc.tile_pool(name="cstg", bufs=3) as cstg,
            tc.tile_pool(name="stg8", bufs=2) as stg8,
            tc.tile_pool(name="wstage", bufs=2) as wstage,
            tc.tile_pool(name="attnT", bufs=2) as attnp,
            tc.tile_pool(name="otp", bufs=2) as otp,
            tc.tile_pool(name="yout", bufs=2) as youtp,
            tc.tile_pool(name="small", bufs=4) as small,
            # one uniform bank-sized ring for transposes, projections,
            # sims and output tiles: 5 slots decouple the front-end from
            # the attention pipeline (5 + 2 acc + 1 den = 8 banks).
            tc.tile_pool(name="ps5", bufs=5, space="PSUM") as ps5,
            tc.tile_pool(name="accp", bufs=2, space="PSUM") as accp,
            tc.tile_pool(name="denp", bufs=1, space="PSUM") as denp,
        ):
            # ---------------- constants ----------------
            ident8 = persist.tile([_P, _P], F8, tag="ident8")
            make_identity(nc, ident8)
            one16 = persist.tile([1, 1], dt.bfloat16, tag="one16")
            nc.vector.memset(one16, 16.0)

            # ---------------- persistent tensors ----------------
            x_nat = persist.tile([_P, _IT, _K], dt.float32, tag="xnat")
            # transposed activations: fp8 data at even bytes (the PE's
            # fp8-transpose writes element step 2), stored as u16 with
            # junk high bytes; consumers use stride-2 fp8 APs.
            xTs = persist.tile([_P, _KT, _I], dt.uint16, tag="xTs")
            cTs = persist.tile([_P, _KT, _J], dt.uint16, tag="cTs")
            xT8 = xTs.bitcast(F8)
            cT8 = cTs.bitcast(F8)

            def pair_ap(t8, kt0, j0, nj):
                """[K=128, 2, nj] fp8 AP over a stride-2 transposed tensor:
                pair dim = (kt0, kt0+1), free dim = nj j's from j0."""
                full = t8[:]
                return bass.AP(
                    tensor=full.tensor,
                    offset=full.offset + kt0 * 2 * _J + 2 * j0,
                    ap=[full.ap[0], [2 * _J, 2], [2, nj]],
                )
            qT = persist.tile([_P, _DT, _I], F8, tag="qT")
            kT = persist.tile([_P, _DT, _J], F8, tag="kT")
            v = persist.tile([_P, _JT, _D], F8, tag="v")
            recips = persist.tile([_P, _IT], dt.float32, tag="recips")

            wq8 = persist.tile([_P, _KT, _D], F8, tag="wq8")
            wk8 = persist.tile([_P, _KT, _D], F8, tag="wk8")
            wv8 = persist.tile([_P, _KT, _D], F8, tag="wv8")
            wo8 = persist.tile([_P, _DT, _K], F8, tag="wo8")
            mask01 = persist.tile([_P, _JT], dt.float32, tag="mask01")
            # mask broadcast along a 128-wide M dim: the denominator runs
            # as an M=128 DoubleRow matmul (M=1 fp8 ldweights is rejected
            # by the ISA's dual-fp8 restrictions); every PSUM row gets the
            # same masked column sum.
            mask8b = persist.tile([_P, _JT, _P], F8, tag="mask8b")
            bo16 = persist.tile([1, _K], dt.bfloat16, tag="bo16")

            # the denominator bank: pden ([1,512] accum) and the 4
            # K=1 transposed-denominator columns share one PSUM bank;
            # every reuse is covered by the pending-zero semantics of
            # start=True plus the strict RAW/WAR ordering on the tile.
            denbank = denp.tile([_P, _NB], dt.float32, tag="denbank")

            # ---------------- weight / mask staging ----------------
            def load_w_early():
                for w_dram, w_sb in ((wk_d, wk8), (wv_d, wv8), (wq_d, wq8)):
                    ws = wstage.tile([_P, _KT, _D], dt.float32, tag="ws",
                                     bufs=3, name=f"ws_{w_sb.name}")
                    nc.sync.dma_start(
                        out=ws, in_=w_dram[:].rearrange("(t p) d -> p t d", p=_P)
                    )
                    if w_sb is wq8:
                        nc.vector.tensor_scalar(
                            out=w_sb, in0=ws, scalar1=_WS, scalar2=None,
                            op0=Alu.mult,
                        )
                    else:
                        nc.scalar.activation(
                            out=w_sb, in_=ws, func=Act.Copy, scale=_WS
                        )
                msk8 = small.tile([_P, _JT], dt.uint8, tag="msk8")
                nc.sync.dma_start(
                    out=msk8, in_=m_d[:].rearrange("(t p) -> p t", p=_P)
                )
                nc.vector.tensor_copy(out=mask01, in_=msk8)
                m01 = mask01[:]
                nc.vector.tensor_copy(
                    out=mask8b,
                    in_=bass.AP(tensor=m01.tensor, offset=m01.offset,
                                ap=[m01.ap[0], [1, _JT], [0, _P]]),
                )

            def load_w_late():
                ws = wstage.tile([_P, _DT, _K], dt.float32, tag="wso")
                nc.sync.dma_start(
                    out=ws, in_=wo_d[:].rearrange("(t p) k -> p t k", p=_P)
                )
                nc.vector.tensor_scalar(
                    out=wo8, in0=ws, scalar1=_WS, scalar2=None, op0=Alu.mult
                )
                bs = small.tile([1, _K], dt.float32, tag="bs")
                nc.sync.dma_start(out=bs, in_=bo_d[:].rearrange("(o k) -> o k", o=1))
                # rank-1 bias term is den (x) 16*bo; *1/(16 den) later
                nc.vector.tensor_scalar(
                    out=bo16, in0=bs, scalar1=16.0, scalar2=None, op0=Alu.mult
                )

            # ---------------- front-end helpers ----------------
            tr_evict_n = [0]

            def transpose_half(src8, rt0, dstTs, col0):
                """8 step-2 fp8 PE transposes (2 row-tiles x 4 kt) packed
                into one PSUM bank; one u16 eviction (2x DVE mode) into
                dstTs[:, :, col0:col0+256]."""
                pk = ps5.tile([_P, _NB], dt.float32, tag="ps")
                pk8 = pk.bitcast(F8)
                for kt in range(_KT):
                    for rt in range(2):
                        slot = kt * 2 + rt
                        sl = pk8[:, slot * 256:(slot + 1) * 256]
                        stepped = bass.AP(
                            tensor=sl.tensor, offset=sl.offset,
                            ap=[sl.ap[0], [2, _P]],
                        )
                        nc.tensor.matmul(
                            stepped,
                            lhsT=src8[:, rt0 + rt, kt * _P:(kt + 1) * _P],
                            rhs=ident8,
                            is_transpose=True,
                            start=(slot == 0), stop=True,
                            skip_group_check=True,
                        )
                # src slots are [kt][rt][j] contiguous = [P, 4, 256] u16;
                # dst is [P, 4(kt), 256] u16 with kt stride 2048.
                dst = dstTs[:, :, col0:col0 + 2 * _P]
                pku = pk.bitcast(dt.uint16)[:]
                src = bass.AP(
                    tensor=pku.tensor, offset=pku.offset,
                    ap=[pku.ap[0], [256, _KT], [1, 256]],
                )
                if tr_evict_n[0] % 2 == 0:
                    nc.vector.tensor_copy(out=dst, in_=src)
                else:
                    nc.scalar.copy(out=dst, in_=src)
                tr_evict_n[0] += 1

            def ctx_group(g):
                cst = cstg.tile([_P, 4, _K], dt.float32, tag="cst")
                if g == 0:
                    splits = ((0, 1), (1, 1), (2, 2))
                else:
                    splits = ((0, 2), (2, 2))
                for o, n in splits:
                    r0 = (g * 4 + o) * _P
                    nc.sync.dma_start(
                        out=cst[:, o:o + n, :],
                        in_=c_d[r0:r0 + n * _P, :].rearrange(
                            "(t p) k -> p t k", p=_P
                        ),
                    )
                c8g = stg8.tile([_P, 4, _K], F8, tag="c8g")
                for rt in range(4):
                    jt = g * 4 + rt
                    # mask folded in: masked j rows of ctx become 0 so
                    # both kT columns and v rows vanish for them.
                    nc.gpsimd.tensor_scalar(
                        out=c8g[:, rt, :], in0=cst[:, rt, :],
                        scalar1=mask01[:, jt:jt + 1], scalar2=None,
                        op0=Alu.mult,
                    )
                transpose_half(c8g, 0, cTs, g * 4 * _P)
                transpose_half(c8g, 2, cTs, g * 4 * _P + 2 * _P)
                # kT projection for this 512-j block, one bank per dh
                for dh in range(_DT):
                    pj = ps5.tile([_P, _NB], dt.float32, tag="ps",
                                  name=f"pk{g}_{dh}")
                    for t in range(2):
                        nc.tensor.matmul(
                            pj,
                            lhsT=wk8[:, 2 * t:2 * t + 2, dh * _P:(dh + 1) * _P],
                            rhs=pair_ap(cT8, 2 * t, g * _NB, _NB),
                            perf_mode=DR, start=(t == 0), stop=(t == 1),
                            skip_group_check=True,
                        )
                    nc.scalar.copy(
                        out=kT[:, dh, g * _NB:(g + 1) * _NB], in_=pj
                    )
                # v projection, two j-tiles packed per PSUM bank
                for jp in range(2):
                    jt0 = g * 4 + 2 * jp
                    pv = ps5.tile([_P, _NB], dt.float32, tag="ps",
                                  name=f"pv{g}_{jp}")
                    for jo in range(2):
                        for t in range(2):
                            nc.tensor.matmul(
                                pv[:, jo * _D:(jo + 1) * _D],
                                lhsT=pair_ap(cT8, 2 * t, (jt0 + jo) * _P, _P),
                                rhs=wv8[:, 2 * t:2 * t + 2, :],
                                perf_mode=DR,
                                start=(jo == 0 and t == 0), stop=(t == 1),
                                skip_group_check=True,
                            )
                    nc.vector.tensor_copy(
                        out=v[:, jt0:jt0 + 2, :],
                        in_=pv[:].rearrange("p (j d) -> p j d", j=2),
                    )

            def x_group(b, dma_only=False):
                r0 = b * 4 * _P
                for h in range(2):
                    nc.sync.dma_start(
                        out=x_nat[:, b * 4 + 2 * h:b * 4 + 2 * h + 2, :],
                        in_=x_d[r0 + 2 * h * _P:r0 + (2 * h + 2) * _P, :]
                        .rearrange("(t p) k -> p t k", p=_P),
                    )

            def x_front(b):
                x8g = stg8.tile([_P, 4, _K], F8, tag="x8g")
                nc.gpsimd.tensor_copy(out=x8g, in_=x_nat[:, b * 4:b * 4 + 4, :])
                transpose_half(x8g, 0, xTs, b * 4 * _P)
                transpose_half(x8g, 2, xTs, b * 4 * _P + 2 * _P)
                for dh in range(_DT):
                    pj = ps5.tile([_P, _NB], dt.float32, tag="ps",
                                  name=f"pq{b}_{dh}")
                    for t in range(2):
                        nc.tensor.matmul(
                            pj,
                            lhsT=wq8[:, 2 * t:2 * t + 2, dh * _P:(dh + 1) * _P],
                            rhs=pair_ap(xT8, 2 * t, b * _NB, _NB),
                            perf_mode=DR, start=(t == 0), stop=(t == 1),
                            skip_group_check=True,
                        )
                    nc.scalar.copy(
                        out=qT[:, dh, b * _NB:(b + 1) * _NB], in_=pj
                    )

            # ---------------- attention ----------------
            aTs = {}
            accs = {}

            def start_block(b):
                aTs[b] = attnp.tile([_P, _JT, _NB], F8, tag="aT",
                                    name=f"aT{b}")
                accs[b] = [
                    accp.tile([_P, _NB], dt.float32, tag="acc",
                              name=f"acc{b}_{dh}")
                    for dh in range(_DT)
                ]

            def sim_exp(b, jt):
                sm = ps5.tile([_P, _NB], dt.float32, tag="ps",
                              name=f"sim{b}_{jt}")
                nc.tensor.matmul(
                    sm,
                    lhsT=kT[:, :, jt * _P:(jt + 1) * _P],
                    rhs=qT[:, :, b * _NB:(b + 1) * _NB],
                    perf_mode=DR, start=True, stop=True,
                    skip_group_check=True,
                )
                dst = aTs[b][:, jt, :]
                if jt % 3 == 2:
                    # Schraudolph fp8 exp on DVE: int8 bits of e4m3
                    nc.vector.tensor_scalar(
                        out=dst.bitcast(dt.int8), in0=sm,
                        scalar1=_SEXP * _SCH_A, scalar2=_SCH_B,
                        op0=Alu.mult, op1=Alu.add,
                    )
                else:
                    nc.scalar.activation(
                        out=dst, in_=sm, func=Act.Exp, bias=0.0, scale=_SEXP
                    )

            def pv_den(b, pr):
                aT = aTs[b]
                for dh in range(_DT):
                    nc.tensor.matmul(
                        accs[b][dh],
                        lhsT=v[:, 2 * pr:2 * pr + 2, dh * _P:(dh + 1) * _P],
                        rhs=aT[:, 2 * pr:2 * pr + 2, :],
                        perf_mode=DR, start=(pr == 0), stop=(pr == 7),
                        skip_group_check=True,
                    )
                nc.tensor.matmul(
                    denbank[:, :],
                    lhsT=mask8b[:, 2 * pr:2 * pr + 2, :],
                    rhs=aT[:, 2 * pr:2 * pr + 2, :],
                    perf_mode=DR, start=(pr == 0), stop=(pr == 7),
                    skip_group_check=True,
                )

            def finish_block(b):
                # oT = accs/64 in fp8 (ScalarE), frees the acc banks
                oTt = otp.tile([_P, _DT, _NB], F8, tag="oT", name=f"oT{b}")
                for dh in range(_DT):
                    nc.scalar.activation(
                        out=oTt[:, dh, :], in_=accs[b][dh],
                        func=Act.Copy, scale=1.0 / 64.0,
                    )
                del accs[b], aTs[b]
                den16 = small.tile([1, _NB], dt.bfloat16, tag="den16",
                                   name=f"den16_{b}")
                nc.vector.tensor_copy(out=den16, in_=denbank[:1, :])
                # transpose den to i-major via K=1 matmuls: trd = 16*den
                for tt in range(4):
                    nc.tensor.matmul(
                        denbank[:, tt:tt + 1],
                        lhsT=den16[:, tt * _P:(tt + 1) * _P],
                        rhs=one16,
                        start=(tt == 0), stop=True,
                        skip_group_check=True,
                    )
                nc.vector.reciprocal(
                    out=recips[:, 4 * b:4 * b + 4], in_=denbank[:, 0:4]
                )
                ys = youtp.tile([_P, 4, _K], dt.float32, tag="ys",
                                name=f"ys{b}")
                for tt in range(4):
                    t = 4 * b + tt
                    yp = ps5.tile([_P, _K], dt.float32, tag="ps",
                                  name=f"yp{t}")
                    nc.tensor.matmul(
                        yp,
                        lhsT=oTt[:, :, tt * _P:(tt + 1) * _P],
                        rhs=wo8,
                        perf_mode=DR, start=True, stop=False,
                        skip_group_check=True,
                    )
                    nc.tensor.matmul(
                        yp,
                        lhsT=den16[:, tt * _P:(tt + 1) * _P],
                        rhs=bo16,
                        start=False, stop=True,
                        skip_group_check=True,
                    )
                    nc.vector.scalar_tensor_tensor(
                        out=ys[:, tt, :], in0=yp, scalar=recips[:, t:t + 1],
                        in1=x_nat[:, t, :], op0=Alu.mult, op1=Alu.add,
                    )
                nc.scalar.dma_start(
                    out=y_d[b * 4 * _P:(b + 1) * 4 * _P, :].rearrange(
                        "(t p) k -> p t k", p=_P
                    ),
                    in_=ys,
                )

            # ---------------- schedule ----------------
            load_w_early()
            x_group(0)
            x_front(0)
            start_block(0)
            for g in range(4):
                ctx_group(g)
                for jt in (4 * g, 4 * g + 1, 4 * g + 2, 4 * g + 3):
                    sim_exp(0, jt)
                    if jt % 2 == 1 and jt >= 3:
                        pv_den(0, (jt - 3) // 2)
            x_group(1)
            load_w_late()
            x_group(2)
            x_group(3)
            pv_den(0, 7)
            finish_block(0)
            for b in range(1, 4):
                x_front(b)
                start_block(b)
                for jt in range(_JT):
                    sim_exp(b, jt)
                    if jt % 2 == 1 and jt >= 3:
                        pv_den(b, (jt - 3) // 2)
                pv_den(b, 7)
                finish_block(b)

    _split_multi_waits(nc, {"*": 1})
    nc.finalize()
    return nc


def kernel(x, context, mask, Wq, Wk, Wv, Wo, bo):
    from concourse.bass_utils import run_bass_kernel_spmd

    if "nc" not in _CACHE:
        _CACHE["nc"] = _build_nc()
    nc = _CACHE["nc"]

    x = np.ascontiguousarray(np.asarray(x, dtype=np.float32))
    context = np.ascontiguousarray(np.asarray(context, dtype=np.float32))
    mask_u8 = np.ascontiguousarray(np.asarray(mask).astype(np.uint8))
    shared = {
        "Wq": np.ascontiguousarray(np.asarray(Wq, dtype=np.float32)),
        "Wk": np.ascontiguousarray(np.asarray(Wk, dtype=np.float32)),
        "Wv": np.ascontiguousarray(np.asarray(Wv, dtype=np.float32)),
        "Wo": np.ascontiguousarray(np.asarray(Wo, dtype=np.float32)),
        "bo": np.ascontiguousarray(np.asarray(bo, dtype=np.float32)),
    }
    in_maps = [
        {"x": x[b], "context": context[b], "mask": mask_u8[b], **shared}
        for b in range(_B)
    ]
    res = run_bass_kernel_spmd(nc, in_maps, core_ids=list(range(_B)))
    return np.stack([res.results[b]["out"] for b in range(_B)], axis=0)


# revision 24
# speedup vs baseline: 1.0190x; 1.0030x over previous
"""Cross-attention Trainium2 kernel (Bass/Tile), SPMD over 8 NeuronCores.

Problem: b=8, i=j=2048, query/context dim 512, inner dim 256.
Sharding: data-parallel over batch - one batch element per core, no
collectives. Each core computes, for its batch element:

    q = x @ Wq ; k = ctx @ Wk ; v = ctx @ Wv
    sim = (q @ k^T) * d^-0.5 ; attn = softmax_j(sim) masked on j
    out = attn @ v ; y = out @ Wo + bo + x

fp8 dataflow (all big matmuls fp8e4m3 DoubleRow: K=256 contracted per
instruction at 0.5 cyc/row):
  1. Weights staged f32 -> fp8 scaled by 32 (keeps N(0,0.02^2) weights
     out of the fp8 denormal range).
  2. x/ctx cast f32 -> fp8 on GpSimd (the context cast folds the mask
     in as a per-j row scale, so masked j contribute exactly zero to
     PV); PE-transposed to k-major, 16 [128,128] tiles packed per PSUM
     bank (start=True only on the bank's first write; later writes
     land on pending-zero bytes), one u32-bitcast eviction per group.
  3. qT/kT (d-major) and v (j-major) projections via DoubleRow pairs;
     PSUM->SBUF evictions cast to fp8 for free.
  4. simT[j,i] per (i-block, jt-pair) -> exp: ScalarE Act.Exp with the
     combined scale (d^-0.5/32^2) writing fp8 directly, a share of
     tiles on DVE via the Schraudolph bit trick (round(x*8/ln2+B) as
     int8 IS fp8e4m3 of e^x to ~5%); denominator = maskT @ attn
     DoubleRow rank-1; both consume the same quantized attn the PV
     matmuls use.
  5. outT accumulated over j in PSUM, evicted /64 to fp8; y per i-tile
     = oT^T @ wo (DoubleRow) + rank-1 den (x) 16*bo (so the bias lands
     pre-normalization exactly); DVE fuses *1/(16 den) + x residual.
DMA: inputs on the SP HWDGE queue ordered [wk wv wq mask, x0, ctx0-3,
x1, wo bo, x2 x3] so attention block 0 streams behind the ctx DMAs;
y writebacks ride the Activation HWDGE queue so they never block
input loads (all transfers serialize on the DMA engines anyway).
"""

import sys

import numpy as np

if "/opt/trn_rl_repo" not in sys.path:
    sys.path.insert(0, "/opt/trn_rl_repo")

_P = 128          # partitions
_B = 8            # batch == number of cores
_I = 2048         # query sequence length
_J = 2048         # context sequence length
_K = 512          # query/context feature dim
_D = 256          # inner dim
_NB = 512         # i-block size
_KT = _K // _P    # 4 contraction tiles for projections
_DT = _D // _P    # 2 inner-dim tiles
_IT = _I // _P    # 16 i tiles
_JT = _J // _P    # 16 j tiles
_WS = 32.0        # weight prescale into fp8 range
# exp argument: sim_psum * d^-0.5 / WS^2
_SEXP = float(_D) ** -0.5 / (_WS * _WS)
# Schraudolph fp8e4m3 exp: bits = round(x * 8/ln2 + 55.63)
_SCH_A = 8.0 / float(np.log(2.0))
_SCH_B = 55.63

_CACHE = {}


def _split_multi_waits(nc, limits):
    """Walrus in this container rejects instructions carrying more sem
    waits than its per-template slot count (e.g. Drain allows 1). Move
    excess waits onto wait-only Drain carriers on the same engine,
    inserted just before the instruction - semantically identical."""
    from concourse import mybir

    n_split = 0
    for func in nc.m.functions:
        for block in func.blocks:
            out = []
            for inst in block.instructions:
                si = inst.sync_info
                maxw = limits.get(type(inst).__name__, limits.get("*"))
                if (
                    maxw is not None
                    and si is not None
                    and si.on_wait
                    and len(si.on_wait) > maxw
                ):
                    waits = list(si.on_wait)
                    keep, rest = waits[:maxw], waits[maxw:]
                    for i in range(0, len(rest), 1):
                        car = mybir.InstDrain(
                            name=f"I-waitcar-{nc.next_id()}", ins=[], outs=[]
                        )
                        car.engine = inst.engine
                        car.sync_info = mybir.SyncInfo(
                            on_wait=[rest[i]], on_update=[]
                        )
                        nc.register_instruction(car)
                        out.append(car)
                        n_split += 1
                    inst.sync_info = mybir.SyncInfo(
                        on_wait=keep, on_update=list(si.on_update or [])
                    )
                out.append(inst)
            block.instructions = out
    return n_split


def _build_nc():
    import concourse.bass as bass
    import concourse.tile as tile
    from concourse import mybir
    from concourse.masks import make_identity

    dt = mybir.dt
    Alu = mybir.AluOpType
    Act = mybir.ActivationFunctionType
    DR = mybir.MatmulPerfMode.DoubleRow
    F8 = dt.float8e4

    nc = bass.Bass("TRN2", target_bir_lowering=False)

    x_d = nc.dram_tensor("x", [_I, _K], dt.float32, kind="ExternalInput")
    c_d = nc.dram_tensor("context", [_J, _K], dt.float32, kind="ExternalInput")
    m_d = nc.dram_tensor("mask", [_J], dt.uint8, kind="ExternalInput")
    wq_d = nc.dram_tensor("Wq", [_K, _D], dt.float32, kind="ExternalInput")
    wk_d = nc.dram_tensor("Wk", [_K, _D], dt.float32, kind="ExternalInput")
    wv_d = nc.dram_tensor("Wv", [_K, _D], dt.float32, kind="ExternalInput")
    wo_d = nc.dram_tensor("Wo", [_D, _K], dt.float32, kind="ExternalInput")
    bo_d = nc.dram_tensor("bo", [_K], dt.float32, kind="ExternalInput")
    y_d = nc.dram_tensor("out", [_I, _K], dt.float32, kind="ExternalOutput")

    with tile.TileContext(nc) as tc:
        with (
            tc.tile_pool(name="persist", bufs=1) as persist,
            tc.tile_pool(name="cstg", bufs=3) as cstg,
            tc.tile_pool(name="stg8", bufs=2) as stg8,
            tc.tile_pool(name="wstage", bufs=2) as wstage,
            tc.tile_pool(name="attnT", bufs=2) as attnp,
            tc.tile_pool(name="otp", bufs=2) as otp,
            tc.tile_pool(name="yout", bufs=2) as youtp,
            tc.tile_pool(name="small", bufs=4) as small,
            # one uniform bank-sized ring for transposes, projections,
            # sims and output tiles: 5 slots decouple the front-end from
            # the attention pipeline (5 + 2 acc + 1 den = 8 banks).
            tc.tile_pool(name="ps5", bufs=5, space="PSUM") as ps5,
            tc.tile_pool(name="accp", bufs=2, space="PSUM") as accp,
            tc.tile_pool(name="denp", bufs=1, space="PSUM") as denp,
        ):
            # ---------------- constants ----------------
            ident8 = persist.tile([_P, _P], F8, tag="ident8")
            make_identity(nc, ident8)
            one16 = persist.tile([1, 1], dt.bfloat16, tag="one16")
            nc.vector.memset(one16, 16.0)

            # ---------------- persistent tensors ----------------
            x_nat = persist.tile([_P, _IT, _K], dt.float32, tag="xnat")
            # transposed activations: fp8 data at even bytes (the PE's
            # fp8-transpose writes element step 2), stored as u16 with
            # junk high bytes; consumers use stride-2 fp8 APs.
            xTs = persist.tile([_P, _KT, _I], dt.uint16, tag="xTs")
            cTs = persist.tile([_P, _KT, _J], dt.uint16, tag="cTs")
            xT8 = xTs.bitcast(F8)
            cT8 = cTs.bitcast(F8)

            def pair_ap(t8, kt0, j0, nj):
                """[K=128, 2, nj] fp8 AP over a stride-2 transposed tensor:
                pair dim = (kt0, kt0+1), free dim = nj j's from j0."""
                full = t8[:]
                return bass.AP(
                    tensor=full.tensor,
                    offset=full.offset + kt0 * 2 * _J + 2 * j0,
                    ap=[full.ap[0], [2 * _J, 2], [2, nj]],
                )
            qT = persist.tile([_P, _DT, _I], F8, tag="qT")
            kT = persist.tile([_P, _DT, _J], F8, tag="kT")
            v = persist.tile([_P, _JT, _D], F8, tag="v")
            recips = persist.tile([_P, _IT], dt.float32, tag="recips")

            wq8 = persist.tile([_P, _KT, _D], F8, tag="wq8")
            wk8 = persist.tile([_P, _KT, _D], F8, tag="wk8")
            wv8 = persist.tile([_P, _KT, _D], F8, tag="wv8")
            wo8 = persist.tile([_P, _DT, _K], F8, tag="wo8")
            mask01 = persist.tile([_P, _JT], dt.float32, tag="mask01")
            # mask broadcast along a 128-wide M dim: the denominator runs
            # as an M=128 DoubleRow matmul (M=1 fp8 ldweights is rejected
            # by the ISA's dual-fp8 restrictions); every PSUM row gets the
            # same masked column sum.
            mask8b = persist.tile([_P, _JT, _P], F8, tag="mask8b")
            bo16 = persist.tile([1, _K], dt.bfloat16, tag="bo16")

            # the denominator bank: pden ([1,512] accum) and the 4
            # K=1 transposed-denominator columns share one PSUM bank;
            # every reuse is covered by the pending-zero semantics of
            # start=True plus the strict RAW/WAR ordering on the tile.
            denbank = denp.tile([_P, _NB], dt.float32, tag="denbank")

            # ---------------- weight / mask staging ----------------
            def load_w_early():
                # critical-path order: mask (gates ctx casts), wk (gates
                # kT), wq (gates qT); wv rides later, behind ctx0/x0.
                msk8 = small.tile([_P, _JT], dt.uint8, tag="msk8")
                nc.sync.dma_start(
                    out=msk8, in_=m_d[:].rearrange("(t p) -> p t", p=_P)
                )
                nc.vector.tensor_copy(out=mask01, in_=msk8)
                m01 = mask01[:]
                nc.vector.tensor_copy(
                    out=mask8b,
                    in_=bass.AP(tensor=m01.tensor, offset=m01.offset,
                                ap=[m01.ap[0], [1, _JT], [0, _P]]),
                )
                for w_dram, w_sb in ((wk_d, wk8), (wq_d, wq8)):
                    ws = wstage.tile([_P, _KT, _D], dt.float32, tag="ws",
                                     bufs=3, name=f"ws_{w_sb.name}")
                    nc.sync.dma_start(
                        out=ws, in_=w_dram[:].rearrange("(t p) d -> p t d", p=_P)
                    )
                    if w_sb is wq8:
                        nc.vector.tensor_scalar(
                            out=w_sb, in0=ws, scalar1=_WS, scalar2=None,
                            op0=Alu.mult,
                        )
                    else:
                        nc.scalar.activation(
                            out=w_sb, in_=ws, func=Act.Copy, scale=_WS
                        )

            def load_wv():
                ws = wstage.tile([_P, _KT, _D], dt.float32, tag="ws",
                                 bufs=3, name="ws_wv8")
                nc.sync.dma_start(
                    out=ws, in_=wv_d[:].rearrange("(t p) d -> p t d", p=_P)
                )
                nc.scalar.activation(out=wv8, in_=ws, func=Act.Copy, scale=_WS)

            def load_w_late():
                ws = wstage.tile([_P, _DT, _K], dt.float32, tag="wso")
                nc.sync.dma_start(
                    out=ws, in_=wo_d[:].rearrange("(t p) k -> p t k", p=_P)
                )
                nc.vector.tensor_scalar(
                    out=wo8, in0=ws, scalar1=_WS, scalar2=None, op0=Alu.mult
                )
                bs = small.tile([1, _K], dt.float32, tag="bs")
                nc.sync.dma_start(out=bs, in_=bo_d[:].rearrange("(o k) -> o k", o=1))
                # rank-1 bias term is den (x) 16*bo; *1/(16 den) later
                nc.vector.tensor_scalar(
                    out=bo16, in0=bs, scalar1=16.0, scalar2=None, op0=Alu.mult
                )

            # ---------------- front-end helpers ----------------
            tr_evict_n = [0]

            def transpose_half(src8, rt0, dstTs, col0):
                """8 step-2 fp8 PE transposes (2 row-tiles x 4 kt) packed
                into one PSUM bank; one u16 eviction (2x DVE mode) into
                dstTs[:, :, col0:col0+256]."""
                pk = ps5.tile([_P, _NB], dt.float32, tag="ps")
                pk8 = pk.bitcast(F8)
                for kt in range(_KT):
                    for rt in range(2):
                        slot = kt * 2 + rt
                        sl = pk8[:, slot * 256:(slot + 1) * 256]
                        stepped = bass.AP(
                            tensor=sl.tensor, offset=sl.offset,
                            ap=[sl.ap[0], [2, _P]],
                        )
                        nc.tensor.matmul(
                            stepped,
                            lhsT=src8[:, rt0 + rt, kt * _P:(kt + 1) * _P],
                            rhs=ident8,
                            is_transpose=True,
                            start=(slot == 0), stop=True,
                            skip_group_check=True,
                        )
                # src slots are [kt][rt][j] contiguous = [P, 4, 256] u16;
                # dst is [P, 4(kt), 256] u16 with kt stride 2048.
                dst = dstTs[:, :, col0:col0 + 2 * _P]
                pku = pk.bitcast(dt.uint16)[:]
                src = bass.AP(
                    tensor=pku.tensor, offset=pku.offset,
                    ap=[pku.ap[0], [256, _KT], [1, 256]],
                )
                nc.vector.tensor_copy(out=dst, in_=src)
                tr_evict_n[0] += 1

            def ctx_group(g):
                cst = cstg.tile([_P, 4, _K], dt.float32, tag="cst")
                if g == 0:
                    splits = ((0, 1), (1, 1), (2, 2))
                else:
                    splits = ((0, 2), (2, 2))
                for o, n in splits:
                    r0 = (g * 4 + o) * _P
                    nc.sync.dma_start(
                        out=cst[:, o:o + n, :],
                        in_=c_d[r0:r0 + n * _P, :].rearrange(
                            "(t p) k -> p t k", p=_P
                        ),
                    )
                c8g = stg8.tile([_P, 4, _K], F8, tag="c8g")
                for rt in range(4):
                    jt = g * 4 + rt
                    # mask folded in: masked j rows of ctx become 0 so
                    # both kT columns and v rows vanish for them.
                    nc.gpsimd.tensor_scalar(
                        out=c8g[:, rt, :], in0=cst[:, rt, :],
                        scalar1=mask01[:, jt:jt + 1], scalar2=None,
                        op0=Alu.mult,
                    )
                transpose_half(c8g, 0, cTs, g * 4 * _P)
                transpose_half(c8g, 2, cTs, g * 4 * _P + 2 * _P)
                # kT projection for this 512-j block, one bank per dh
                for dh in range(_DT):
                    pj = ps5.tile([_P, _NB], dt.float32, tag="ps",
                                  name=f"pk{g}_{dh}")
                    for t in range(2):
                        nc.tensor.matmul(
                            pj,
                            lhsT=wk8[:, 2 * t:2 * t + 2, dh * _P:(dh + 1) * _P],
                            rhs=pair_ap(cT8, 2 * t, g * _NB, _NB),
                            perf_mode=DR, start=(t == 0), stop=(t == 1),
                            skip_group_check=True,
                        )
                    nc.scalar.copy(
                        out=kT[:, dh, g * _NB:(g + 1) * _NB], in_=pj
                    )

            def vproj(g):
                # v projection, two j-tiles packed per PSUM bank
                for jp in range(2):
                    jt0 = g * 4 + 2 * jp
                    pv = ps5.tile([_P, _NB], dt.float32, tag="ps",
                                  name=f"pv{g}_{jp}")
                    for jo in range(2):
                        for t in range(2):
                            nc.tensor.matmul(
                                pv[:, jo * _D:(jo + 1) * _D],
                                lhsT=pair_ap(cT8, 2 * t, (jt0 + jo) * _P, _P),
                                rhs=wv8[:, 2 * t:2 * t + 2, :],
                                perf_mode=DR,
                                start=(jo == 0 and t == 0), stop=(t == 1),
                                skip_group_check=True,
                            )
                    nc.vector.tensor_copy(
                        out=v[:, jt0:jt0 + 2, :],
                        in_=pv[:].rearrange("p (j d) -> p j d", j=2),
                    )

            def x_group(b, dma_only=False):
                r0 = b * 4 * _P
                for h in range(2):
                    nc.sync.dma_start(
                        out=x_nat[:, b * 4 + 2 * h:b * 4 + 2 * h + 2, :],
                        in_=x_d[r0 + 2 * h * _P:r0 + (2 * h + 2) * _P, :]
                        .rearrange("(t p) k -> p t k", p=_P),
                    )

            def x_front(b):
                x8g = stg8.tile([_P, 4, _K], F8, tag="x8g")
                nc.gpsimd.tensor_copy(out=x8g, in_=x_nat[:, b * 4:b * 4 + 4, :])
                transpose_half(x8g, 0, xTs, b * 4 * _P)
                transpose_half(x8g, 2, xTs, b * 4 * _P + 2 * _P)
                for dh in range(_DT):
                    pj = ps5.tile([_P, _NB], dt.float32, tag="ps",
                                  name=f"pq{b}_{dh}")
                    for t in range(2):
                        nc.tensor.matmul(
                            pj,
                            lhsT=wq8[:, 2 * t:2 * t + 2, dh * _P:(dh + 1) * _P],
                            rhs=pair_ap(xT8, 2 * t, b * _NB, _NB),
                            perf_mode=DR, start=(t == 0), stop=(t == 1),
                            skip_group_check=True,
                        )
                    nc.scalar.copy(
                        out=qT[:, dh, b * _NB:(b + 1) * _NB], in_=pj
                    )

            # ---------------- attention ----------------
            aTs = {}
            accs = {}

            def start_block(b):
                aTs[b] = attnp.tile([_P, _JT, _NB], F8, tag="aT",
                                    name=f"aT{b}")
                accs[b] = [
                    accp.tile([_P, _NB], dt.float32, tag="acc",
                              name=f"acc{b}_{dh}")
                    for dh in range(_DT)
                ]

            def sim_exp(b, jt):
                sm = ps5.tile([_P, _NB], dt.float32, tag="ps",
                              name=f"sim{b}_{jt}")
                nc.tensor.matmul(
                    sm,
                    lhsT=kT[:, :, jt * _P:(jt + 1) * _P],
                    rhs=qT[:, :, b * _NB:(b + 1) * _NB],
                    perf_mode=DR, start=True, stop=True,
                    skip_group_check=True,
                )
                dst = aTs[b][:, jt, :]
                if jt % 3 == 2:
                    # Schraudolph fp8 exp on DVE: int8 bits of e4m3
                    nc.vector.tensor_scalar(
                        out=dst.bitcast(dt.int8), in0=sm,
                        scalar1=_SEXP * _SCH_A, scalar2=_SCH_B,
                        op0=Alu.mult, op1=Alu.add,
                    )
                else:
                    nc.scalar.activation(
                        out=dst, in_=sm, func=Act.Exp, bias=0.0, scale=_SEXP
                    )

            def pv_den(b, pr):
                aT = aTs[b]
                for dh in range(_DT):
                    nc.tensor.matmul(
                        accs[b][dh],
                        lhsT=v[:, 2 * pr:2 * pr + 2, dh * _P:(dh + 1) * _P],
                        rhs=aT[:, 2 * pr:2 * pr + 2, :],
                        perf_mode=DR, start=(pr == 0), stop=(pr == 7),
                        skip_group_check=True,
                    )
                nc.tensor.matmul(
                    denbank[:, :],
                    lhsT=mask8b[:, 2 * pr:2 * pr + 2, :],
                    rhs=aT[:, 2 * pr:2 * pr + 2, :],
                    perf_mode=DR, start=(pr == 0), stop=(pr == 7),
                    skip_group_check=True,
                )

            def finish_block(b):
                # oT = accs/64 in fp8 (ScalarE), frees the acc banks
                oTt = otp.tile([_P, _DT, _NB], F8, tag="oT", name=f"oT{b}")
                for dh in range(_DT):
                    nc.scalar.activation(
                        out=oTt[:, dh, :], in_=accs[b][dh],
                        func=Act.Copy, scale=1.0 / 64.0,
                    )
                del accs[b], aTs[b]
                den16 = small.tile([1, _NB], dt.bfloat16, tag="den16",
                                   name=f"den16_{b}")
                nc.vector.tensor_copy(out=den16, in_=denbank[:1, :])
                # transpose den to i-major via K=1 matmuls: trd = 16*den
                for tt in range(4):
                    nc.tensor.matmul(
                        denbank[:, tt:tt + 1],
                        lhsT=den16[:, tt * _P:(tt + 1) * _P],
                        rhs=one16,
                        start=(tt == 0), stop=True,
                        skip_group_check=True,
                    )
                nc.vector.reciprocal(
                    out=recips[:, 4 * b:4 * b + 4], in_=denbank[:, 0:4]
                )
                ys = youtp.tile([_P, 4, _K], dt.float32, tag="ys",
                                name=f"ys{b}")
                for tt in range(4):
                    t = 4 * b + tt
                    yp = ps5.tile([_P, _K], dt.float32, tag="ps",
                                  name=f"yp{t}")
                    nc.tensor.matmul(
                        yp,
                        lhsT=oTt[:, :, tt * _P:(tt + 1) * _P],
                        rhs=wo8,
                        perf_mode=DR, start=True, stop=False,
                        skip_group_check=True,
                    )
                    nc.tensor.matmul(
                        yp,
                        lhsT=den16[:, tt * _P:(tt + 1) * _P],
                        rhs=bo16,
                        start=False, stop=True,
                        skip_group_check=True,
                    )
                    nc.vector.scalar_tensor_tensor(
                        out=ys[:, tt, :], in0=yp, scalar=recips[:, t:t + 1],
                        in1=x_nat[:, t, :], op0=Alu.mult, op1=Alu.add,
                    )
                nc.sync.dma_start(
                    out=y_d[b * 4 * _P:(b + 1) * 4 * _P, :].rearrange(
                        "(t p) k -> p t k", p=_P
                    ),
                    in_=ys,
                )

            # ---------------- schedule ----------------
            # DMA order: mask wk wq | x0 ctx0 | wv ctx1 ctx2 ctx3 |
            # x1 x2 x3 wo bo | y0..y3. Block-0 attention streams behind
            # the ctx groups; v-projections lag one group behind (wv
            # arrives after ctx0).
            load_w_early()
            x_group(0)
            x_front(0)
            start_block(0)
            for g in range(4):
                ctx_group(g)
                if g == 0:
                    load_wv()
                else:
                    vproj(g - 1)
                if g == 3:
                    x_group(1)
                for jt in (4 * g, 4 * g + 1, 4 * g + 2, 4 * g + 3):
                    sim_exp(0, jt)
                # PV lags vproj by one ctx group
                if g >= 1:
                    pv_den(0, 2 * (g - 1))
                    pv_den(0, 2 * (g - 1) + 1)
            vproj(3)
            x_group(2)
            x_group(3)
            load_w_late()
            for pr in (4, 5, 6, 7):
                pv_den(0, pr)
            finish_block(0)
            for b in range(1, 4):
                x_front(b)
                start_block(b)
                for jt in range(_JT):
                    sim_exp(b, jt)
                    if jt % 2 == 1 and jt >= 3:
                        pv_den(b, (jt - 3) // 2)
                pv_den(b, 7)
                finish_block(b)

    _split_multi_waits(nc, {"*": 1})
    nc.finalize()
    return nc


def kernel(x, context, mask, Wq, Wk, Wv, Wo, bo):
    from concourse.bass_utils import run_bass_kernel_spmd

    if "nc" not in _CACHE:
        _CACHE["nc"] = _build_nc()
    nc = _CACHE["nc"]

    x = np.ascontiguousarray(np.asarray(x, dtype=np.float32))
    context = np.ascontiguousarray(np.asarray(context, dtype=np.float32))
    mask_u8 = np.ascontiguousarray(np.asarray(mask).astype(np.uint8))
    shared = {
        "Wq": np.ascontiguousarray(np.asarray(Wq, dtype=np.float32)),
        "Wk": np.ascontiguousarray(np.asarray(Wk, dtype=np.float32)),
        "Wv": np.ascontiguousarray(np.asarray(Wv, dtype=np.float32)),
        "Wo": np.ascontiguousarray(np.asarray(Wo, dtype=np.float32)),
        "bo": np.ascontiguousarray(np.asarray(bo, dtype=np.float32)),
    }
    in_maps = [
        {"x": x[b], "context": context[b], "mask": mask_u8[b], **shared}
        for b in range(_B)
    ]
    res = run_bass_kernel_spmd(nc, in_maps, core_ids=list(range(_B)))
    return np.stack([res.results[b]["out"] for b in range(_B)], axis=0)


# revision 25
# speedup vs baseline: 1.0871x; 1.0668x over previous
"""Cross-attention Trainium2 kernel (Bass/Tile), SPMD over 8 NeuronCores.

Problem: b=8, i=j=2048, query/context dim 512, inner dim 256.
Sharding: data-parallel over batch - one batch element per core, no
collectives. Each core computes, for its batch element:

    q = x @ Wq ; k = ctx @ Wk ; v = ctx @ Wv
    sim = (q @ k^T) * d^-0.5 ; attn = softmax_j(sim) masked on j
    out = attn @ v ; y = out @ Wo + bo + x

fp8 dataflow (all big matmuls fp8e4m3 DoubleRow: K=256 contracted per
instruction at 0.5 cyc/row):
  1. Weights staged f32 -> fp8 scaled by 32 (keeps N(0,0.02^2) weights
     out of the fp8 denormal range).
  2. x/ctx cast f32 -> fp8 on GpSimd (the context cast folds the mask
     in as a per-j row scale, so masked j contribute exactly zero to
     PV); PE-transposed to k-major, 16 [128,128] tiles packed per PSUM
     bank (start=True only on the bank's first write; later writes
     land on pending-zero bytes), one u32-bitcast eviction per group.
  3. qT/kT (d-major) and v (j-major) projections via DoubleRow pairs;
     PSUM->SBUF evictions cast to fp8 for free.
  4. simT[j,i] per (i-block, jt-pair) -> exp: ScalarE Act.Exp with the
     combined scale (d^-0.5/32^2) writing fp8 directly, a share of
     tiles on DVE via the Schraudolph bit trick (round(x*8/ln2+B) as
     int8 IS fp8e4m3 of e^x to ~5%); denominator = maskT @ attn
     DoubleRow rank-1; both consume the same quantized attn the PV
     matmuls use.
  5. outT accumulated over j in PSUM, evicted /64 to fp8; y per i-tile
     = oT^T @ wo (DoubleRow) + rank-1 den (x) 16*bo (so the bias lands
     pre-normalization exactly); DVE fuses *1/(16 den) + x residual.
DMA: inputs on the SP HWDGE queue ordered [wk wv wq mask, x0, ctx0-3,
x1, wo bo, x2 x3] so attention block 0 streams behind the ctx DMAs;
y writebacks ride the Activation HWDGE queue so they never block
input loads (all transfers serialize on the DMA engines anyway).
"""

import sys

import numpy as np

if "/opt/trn_rl_repo" not in sys.path:
    sys.path.insert(0, "/opt/trn_rl_repo")

_P = 128          # partitions
_B = 8            # batch == number of cores
_I = 2048         # query sequence length
_J = 2048         # context sequence length
_K = 512          # query/context feature dim
_D = 256          # inner dim
_NB = 512         # i-block size
_KT = _K // _P    # 4 contraction tiles for projections
_DT = _D // _P    # 2 inner-dim tiles
_IT = _I // _P    # 16 i tiles
_JT = _J // _P    # 16 j tiles
_WS = 32.0        # weight prescale into fp8 range
# exp argument: sim_psum * d^-0.5 / WS^2
_SEXP = float(_D) ** -0.5 / (_WS * _WS)
# Schraudolph fp8e4m3 exp: bits = round(x * 8/ln2 + 55.63)
_SCH_A = 8.0 / float(np.log(2.0))
_SCH_B = 55.63

_CACHE = {}


def _split_multi_waits(nc, limits):
    """Walrus in this container rejects instructions carrying more sem
    waits than its per-template slot count (e.g. Drain allows 1). Move
    excess waits onto wait-only Drain carriers on the same engine,
    inserted just before the instruction - semantically identical."""
    from concourse import mybir

    n_split = 0
    for func in nc.m.functions:
        for block in func.blocks:
            out = []
            for inst in block.instructions:
                si = inst.sync_info
                maxw = limits.get(type(inst).__name__, limits.get("*"))
                if (
                    maxw is not None
                    and si is not None
                    and si.on_wait
                    and len(si.on_wait) > maxw
                ):
                    waits = list(si.on_wait)
                    keep, rest = waits[:maxw], waits[maxw:]
                    for i in range(0, len(rest), 1):
                        car = mybir.InstDrain(
                            name=f"I-waitcar-{nc.next_id()}", ins=[], outs=[]
                        )
                        car.engine = inst.engine
                        car.sync_info = mybir.SyncInfo(
                            on_wait=[rest[i]], on_update=[]
                        )
                        nc.register_instruction(car)
                        out.append(car)
                        n_split += 1
                    inst.sync_info = mybir.SyncInfo(
                        on_wait=keep, on_update=list(si.on_update or [])
                    )
                out.append(inst)
            block.instructions = out
    return n_split


def _build_nc():
    import concourse.bass as bass
    import concourse.tile as tile
    from concourse import mybir
    from concourse.masks import make_identity

    dt = mybir.dt
    Alu = mybir.AluOpType
    Act = mybir.ActivationFunctionType
    DR = mybir.MatmulPerfMode.DoubleRow
    F8 = dt.float8e4

    nc = bass.Bass("TRN2", target_bir_lowering=False)

    x_d = nc.dram_tensor("x", [_I, _K], dt.float32, kind="ExternalInput")
    c_d = nc.dram_tensor("context", [_J, _K], dt.float32, kind="ExternalInput")
    m_d = nc.dram_tensor("mask", [_J], dt.uint8, kind="ExternalInput")
    wq_d = nc.dram_tensor("Wq", [_K, _D], dt.float32, kind="ExternalInput")
    wk_d = nc.dram_tensor("Wk", [_K, _D], dt.float32, kind="ExternalInput")
    wv_d = nc.dram_tensor("Wv", [_K, _D], dt.float32, kind="ExternalInput")
    wo_d = nc.dram_tensor("Wo", [_D, _K], dt.float32, kind="ExternalInput")
    bo_d = nc.dram_tensor("bo", [_K], dt.float32, kind="ExternalInput")
    y_d = nc.dram_tensor("out", [_I, _K], dt.float32, kind="ExternalOutput")

    with tile.TileContext(nc) as tc:
        with (
            tc.tile_pool(name="persist", bufs=1) as persist,
            tc.tile_pool(name="cstg", bufs=3) as cstg,
            tc.tile_pool(name="stg8", bufs=2) as stg8,
            tc.tile_pool(name="wstage", bufs=2) as wstage,
            tc.tile_pool(name="attnT", bufs=2) as attnp,
            tc.tile_pool(name="otp", bufs=2) as otp,
            tc.tile_pool(name="yout", bufs=2) as youtp,
            tc.tile_pool(name="small", bufs=4) as small,
            # one uniform bank-sized ring for transposes, projections,
            # sims and output tiles: 5 slots decouple the front-end from
            # the attention pipeline (5 + 2 acc + 1 den = 8 banks).
            tc.tile_pool(name="ps5", bufs=5, space="PSUM") as ps5,
            tc.tile_pool(name="accp", bufs=2, space="PSUM") as accp,
            tc.tile_pool(name="denp", bufs=1, space="PSUM") as denp,
        ):
            # ---------------- constants ----------------
            ident8 = persist.tile([_P, _P], F8, tag="ident8")
            make_identity(nc, ident8)
            one16 = persist.tile([1, 1], dt.bfloat16, tag="one16")
            nc.vector.memset(one16, 16.0)

            # ---------------- persistent tensors ----------------
            x_nat = persist.tile([_P, _IT, _K], dt.float32, tag="xnat")
            # transposed activations: fp8 data at even bytes (the PE's
            # fp8-transpose writes element step 2), stored as u16 with
            # junk high bytes; consumers use stride-2 fp8 APs.
            xTs = persist.tile([_P, _KT, _I], dt.uint16, tag="xTs")
            cTs = persist.tile([_P, _KT, _J], dt.uint16, tag="cTs")
            xT8 = xTs.bitcast(F8)
            cT8 = cTs.bitcast(F8)

            def pair_ap(t8, kt0, j0, nj):
                """[K=128, 2, nj] fp8 AP over a stride-2 transposed tensor:
                pair dim = (kt0, kt0+1), free dim = nj j's from j0."""
                full = t8[:]
                return bass.AP(
                    tensor=full.tensor,
                    offset=full.offset + kt0 * 2 * _J + 2 * j0,
                    ap=[full.ap[0], [2 * _J, 2], [2, nj]],
                )
            qT = persist.tile([_P, _DT, _I], F8, tag="qT")
            kT = persist.tile([_P, _DT, _J], F8, tag="kT")
            v = persist.tile([_P, _JT, _D], F8, tag="v")
            recips = persist.tile([_P, _IT], dt.float32, tag="recips")

            wq8 = persist.tile([_P, _KT, _D], F8, tag="wq8")
            wk8 = persist.tile([_P, _KT, _D], F8, tag="wk8")
            wv8 = persist.tile([_P, _KT, _D], F8, tag="wv8")
            wo8 = persist.tile([_P, _DT, _K], F8, tag="wo8")
            mask01 = persist.tile([_P, _JT], dt.float32, tag="mask01")
            # mask broadcast along a 128-wide M dim: the denominator runs
            # as an M=128 DoubleRow matmul (M=1 fp8 ldweights is rejected
            # by the ISA's dual-fp8 restrictions); every PSUM row gets the
            # same masked column sum.
            mask8b = persist.tile([_P, _JT, _P], F8, tag="mask8b")
            bo16 = persist.tile([1, _K], dt.bfloat16, tag="bo16")

            # the denominator bank: pden ([1,512] accum) and the 4
            # K=1 transposed-denominator columns share one PSUM bank;
            # every reuse is covered by the pending-zero semantics of
            # start=True plus the strict RAW/WAR ordering on the tile.
            denbank = denp.tile([_P, _NB], dt.float32, tag="denbank")

            # ---------------- weight / mask staging ----------------
            def load_w_early():
                # critical-path order: mask (gates ctx casts), wk (gates
                # kT), wq (gates qT); wv rides later, behind ctx0/x0.
                msk8 = small.tile([_P, _JT], dt.uint8, tag="msk8")
                nc.sync.dma_start(
                    out=msk8, in_=m_d[:].rearrange("(t p) -> p t", p=_P)
                )
                nc.vector.tensor_copy(out=mask01, in_=msk8)
                m01 = mask01[:]
                nc.vector.tensor_copy(
                    out=mask8b,
                    in_=bass.AP(tensor=m01.tensor, offset=m01.offset,
                                ap=[m01.ap[0], [1, _JT], [0, _P]]),
                )
                for w_dram, w_sb in ((wk_d, wk8), (wq_d, wq8)):
                    ws = wstage.tile([_P, _KT, _D], dt.float32, tag="ws",
                                     bufs=3, name=f"ws_{w_sb.name}")
                    nc.sync.dma_start(
                        out=ws, in_=w_dram[:].rearrange("(t p) d -> p t d", p=_P)
                    )
                    if w_sb is wq8:
                        nc.vector.tensor_scalar(
                            out=w_sb, in0=ws, scalar1=_WS, scalar2=None,
                            op0=Alu.mult,
                        )
                    else:
                        nc.scalar.activation(
                            out=w_sb, in_=ws, func=Act.Copy, scale=_WS
                        )

            def load_wv():
                ws = wstage.tile([_P, _KT, _D], dt.float32, tag="ws",
                                 bufs=3, name="ws_wv8")
                nc.sync.dma_start(
                    out=ws, in_=wv_d[:].rearrange("(t p) d -> p t d", p=_P)
                )
                nc.scalar.activation(out=wv8, in_=ws, func=Act.Copy, scale=_WS)

            def load_w_late():
                ws = wstage.tile([_P, _DT, _K], dt.float32, tag="wso")
                nc.sync.dma_start(
                    out=ws, in_=wo_d[:].rearrange("(t p) k -> p t k", p=_P)
                )
                nc.vector.tensor_scalar(
                    out=wo8, in0=ws, scalar1=_WS, scalar2=None, op0=Alu.mult
                )
                bs = small.tile([1, _K], dt.float32, tag="bs")
                nc.sync.dma_start(out=bs, in_=bo_d[:].rearrange("(o k) -> o k", o=1))
                # rank-1 bias term is den (x) 16*bo; *1/(16 den) later
                nc.vector.tensor_scalar(
                    out=bo16, in0=bs, scalar1=16.0, scalar2=None, op0=Alu.mult
                )

            # ---------------- front-end helpers ----------------
            tr_evict_n = [0]

            def transpose_half(src8, rt0, dstTs, col0):
                """8 step-2 fp8 PE transposes (2 row-tiles x 4 kt) packed
                into one PSUM bank; one u16 eviction (2x DVE mode) into
                dstTs[:, :, col0:col0+256]."""
                pk = ps5.tile([_P, _NB], dt.float32, tag="ps")
                pk8 = pk.bitcast(F8)
                for kt in range(_KT):
                    for rt in range(2):
                        slot = kt * 2 + rt
                        sl = pk8[:, slot * 256:(slot + 1) * 256]
                        stepped = bass.AP(
                            tensor=sl.tensor, offset=sl.offset,
                            ap=[sl.ap[0], [2, _P]],
                        )
                        nc.tensor.matmul(
                            stepped,
                            lhsT=src8[:, rt0 + rt, kt * _P:(kt + 1) * _P],
                            rhs=ident8,
                            is_transpose=True,
                            start=(slot == 0), stop=True,
                            skip_group_check=True,
                        )
                # src slots are [kt][rt][j] contiguous = [P, 4, 256] u16;
                # dst is [P, 4(kt), 256] u16 with kt stride 2048.
                dst = dstTs[:, :, col0:col0 + 2 * _P]
                pku = pk.bitcast(dt.uint16)[:]
                src = bass.AP(
                    tensor=pku.tensor, offset=pku.offset,
                    ap=[pku.ap[0], [256, _KT], [1, 256]],
                )
                nc.vector.tensor_copy(out=dst, in_=src)
                tr_evict_n[0] += 1

            def ctx_group(g):
                cst = cstg.tile([_P, 4, _K], dt.float32, tag="cst")
                if g == 0:
                    splits = ((0, 1), (1, 1), (2, 2))
                else:
                    splits = ((0, 2), (2, 2))
                for o, n in splits:
                    r0 = (g * 4 + o) * _P
                    nc.sync.dma_start(
                        out=cst[:, o:o + n, :],
                        in_=c_d[r0:r0 + n * _P, :].rearrange(
                            "(t p) k -> p t k", p=_P
                        ),
                    )
                c8g = stg8.tile([_P, 4, _K], F8, tag="c8g")
                for rt in range(4):
                    jt = g * 4 + rt
                    # mask folded in: masked j rows of ctx become 0 so
                    # both kT columns and v rows vanish for them.
                    nc.gpsimd.tensor_scalar(
                        out=c8g[:, rt, :], in0=cst[:, rt, :],
                        scalar1=mask01[:, jt:jt + 1], scalar2=None,
                        op0=Alu.mult,
                    )
                transpose_half(c8g, 0, cTs, g * 4 * _P)
                transpose_half(c8g, 2, cTs, g * 4 * _P + 2 * _P)
                # kT projection for this 512-j block, one bank per dh
                for dh in range(_DT):
                    pj = ps5.tile([_P, _NB], dt.float32, tag="ps",
                                  name=f"pk{g}_{dh}")
                    for t in range(2):
                        nc.tensor.matmul(
                            pj,
                            lhsT=wk8[:, 2 * t:2 * t + 2, dh * _P:(dh + 1) * _P],
                            rhs=pair_ap(cT8, 2 * t, g * _NB, _NB),
                            perf_mode=DR, start=(t == 0), stop=(t == 1),
                            skip_group_check=True,
                        )
                    nc.scalar.copy(
                        out=kT[:, dh, g * _NB:(g + 1) * _NB], in_=pj
                    )

            def vproj(g):
                # v projection, two j-tiles packed per PSUM bank
                for jp in range(2):
                    jt0 = g * 4 + 2 * jp
                    pv = ps5.tile([_P, _NB], dt.float32, tag="ps",
                                  name=f"pv{g}_{jp}")
                    for jo in range(2):
                        for t in range(2):
                            nc.tensor.matmul(
                                pv[:, jo * _D:(jo + 1) * _D],
                                lhsT=pair_ap(cT8, 2 * t, (jt0 + jo) * _P, _P),
                                rhs=wv8[:, 2 * t:2 * t + 2, :],
                                perf_mode=DR,
                                start=(jo == 0 and t == 0), stop=(t == 1),
                                skip_group_check=True,
                            )
                    nc.vector.tensor_copy(
                        out=v[:, jt0:jt0 + 2, :],
                        in_=pv[:].rearrange("p (j d) -> p j d", j=2),
                    )

            def x_group(b, dma_only=False):
                r0 = b * 4 * _P
                for h in range(2):
                    nc.sync.dma_start(
                        out=x_nat[:, b * 4 + 2 * h:b * 4 + 2 * h + 2, :],
                        in_=x_d[r0 + 2 * h * _P:r0 + (2 * h + 2) * _P, :]
                        .rearrange("(t p) k -> p t k", p=_P),
                    )

            def x_front(b):
                x8g = stg8.tile([_P, 4, _K], F8, tag="x8g")
                nc.gpsimd.tensor_copy(out=x8g, in_=x_nat[:, b * 4:b * 4 + 4, :])
                transpose_half(x8g, 0, xTs, b * 4 * _P)
                transpose_half(x8g, 2, xTs, b * 4 * _P + 2 * _P)
                for dh in range(_DT):
                    pj = ps5.tile([_P, _NB], dt.float32, tag="ps",
                                  name=f"pq{b}_{dh}")
                    for t in range(2):
                        nc.tensor.matmul(
                            pj,
                            lhsT=wq8[:, 2 * t:2 * t + 2, dh * _P:(dh + 1) * _P],
                            rhs=pair_ap(xT8, 2 * t, b * _NB, _NB),
                            perf_mode=DR, start=(t == 0), stop=(t == 1),
                            skip_group_check=True,
                        )
                    nc.scalar.copy(
                        out=qT[:, dh, b * _NB:(b + 1) * _NB], in_=pj
                    )

            # ---------------- attention ----------------
            aTs = {}
            accs = {}

            def start_block(b):
                aTs[b] = attnp.tile([_P, _JT, _NB], F8, tag="aT",
                                    name=f"aT{b}")
                accs[b] = [
                    accp.tile([_P, _NB], dt.float32, tag="acc",
                              name=f"acc{b}_{dh}")
                    for dh in range(_DT)
                ]

            def sim_exp(b, jt):
                sm = ps5.tile([_P, _NB], dt.float32, tag="ps",
                              name=f"sim{b}_{jt}")
                nc.tensor.matmul(
                    sm,
                    lhsT=kT[:, :, jt * _P:(jt + 1) * _P],
                    rhs=qT[:, :, b * _NB:(b + 1) * _NB],
                    perf_mode=DR, start=True, stop=True,
                    skip_group_check=True,
                )
                dst = aTs[b][:, jt, :]
                if jt % 3 == 2:
                    # Schraudolph fp8 exp on DVE: int8 bits of e4m3
                    nc.vector.tensor_scalar(
                        out=dst.bitcast(dt.int8), in0=sm,
                        scalar1=_SEXP * _SCH_A, scalar2=_SCH_B,
                        op0=Alu.mult, op1=Alu.add,
                    )
                else:
                    nc.scalar.activation(
                        out=dst, in_=sm, func=Act.Exp, bias=0.0, scale=_SEXP
                    )

            def pv_den(b, pr):
                aT = aTs[b]
                for dh in range(_DT):
                    nc.tensor.matmul(
                        accs[b][dh],
                        lhsT=v[:, 2 * pr:2 * pr + 2, dh * _P:(dh + 1) * _P],
                        rhs=aT[:, 2 * pr:2 * pr + 2, :],
                        perf_mode=DR, start=(pr == 0), stop=(pr == 7),
                        skip_group_check=True,
                    )
                nc.tensor.matmul(
                    denbank[:, :],
                    lhsT=mask8b[:, 2 * pr:2 * pr + 2, :],
                    rhs=aT[:, 2 * pr:2 * pr + 2, :],
                    perf_mode=DR, start=(pr == 0), stop=(pr == 7),
                    skip_group_check=True,
                )

            def finish_block(b):
                # oT = accs/64 in fp8 (ScalarE), frees the acc banks
                oTt = otp.tile([_P, _DT, _NB], F8, tag="oT", name=f"oT{b}")
                for dh in range(_DT):
                    nc.scalar.activation(
                        out=oTt[:, dh, :], in_=accs[b][dh],
                        func=Act.Copy, scale=1.0 / 64.0,
                    )
                del accs[b], aTs[b]
                den16 = small.tile([1, _NB], dt.bfloat16, tag="den16",
                                   name=f"den16_{b}")
                nc.vector.tensor_copy(out=den16, in_=denbank[:1, :])
                # transpose den to i-major via K=1 matmuls: trd = 16*den
                for tt in range(4):
                    nc.tensor.matmul(
                        denbank[:, tt:tt + 1],
                        lhsT=den16[:, tt * _P:(tt + 1) * _P],
                        rhs=one16,
                        start=(tt == 0), stop=True,
                        skip_group_check=True,
                    )
                nc.vector.reciprocal(
                    out=recips[:, 4 * b:4 * b + 4], in_=denbank[:, 0:4]
                )
                ys = youtp.tile([_P, 4, _K], dt.float32, tag="ys",
                                name=f"ys{b}")
                for tt in range(4):
                    t = 4 * b + tt
                    yp = ps5.tile([_P, _K], dt.float32, tag="ps",
                                  name=f"yp{t}")
                    nc.tensor.matmul(
                        yp,
                        lhsT=oTt[:, :, tt * _P:(tt + 1) * _P],
                        rhs=wo8,
                        perf_mode=DR, start=True, stop=False,
                        skip_group_check=True,
                    )
                    nc.tensor.matmul(
                        yp,
                        lhsT=den16[:, tt * _P:(tt + 1) * _P],
                        rhs=bo16,
                        start=False, stop=True,
                        skip_group_check=True,
                    )
                    nc.vector.scalar_tensor_tensor(
                        out=ys[:, tt, :], in0=yp, scalar=recips[:, t:t + 1],
                        in1=x_nat[:, t, :], op0=Alu.mult, op1=Alu.add,
                    )
                nc.sync.dma_start(
                    out=y_d[b * 4 * _P:(b + 1) * 4 * _P, :].rearrange(
                        "(t p) k -> p t k", p=_P
                    ),
                    in_=ys,
                )

            # ---------------- schedule ----------------
            # DMA order: mask wk wq | x0 ctx0 | wv ctx1 ctx2 ctx3 |
            # x1 x2 x3 wo bo | y0..y3. Block-0 attention streams behind
            # the ctx groups; v-projections lag one group behind (wv
            # arrives after ctx0).
            load_w_early()
            x_group(0)
            x_front(0)
            start_block(0)
            for g in range(4):
                ctx_group(g)
                if g == 0:
                    load_wv()
                else:
                    vproj(g - 1)
                if g == 1:
                    x_group(1)
                for jt in (4 * g, 4 * g + 1, 4 * g + 2, 4 * g + 3):
                    sim_exp(0, jt)
                # PV lags vproj by one ctx group
                if g >= 1:
                    pv_den(0, 2 * (g - 1))
                    pv_den(0, 2 * (g - 1) + 1)
                if g == 2:
                    # x1 landed during g=2's span; prepare qT(1) so
                    # block 1 starts the moment block 0's stream drains
                    x_front(1)
            vproj(3)
            x_group(2)
            x_group(3)
            load_w_late()
            for pr in (4, 5, 6, 7):
                pv_den(0, pr)
            finish_block(0)
            for b in range(1, 4):
                start_block(b)
                for jt in range(_JT):
                    sim_exp(b, jt)
                    if jt == 6 and b < 3:
                        # prepare the next block's qT mid-stream
                        x_front(b + 1)
                    if jt % 2 == 1 and jt >= 3:
                        pv_den(b, (jt - 3) // 2)
                pv_den(b, 7)
                finish_block(b)

    _split_multi_waits(nc, {"*": 1})
    nc.finalize()
    return nc


def kernel(x, context, mask, Wq, Wk, Wv, Wo, bo):
    from concourse.bass_utils import run_bass_kernel_spmd

    if "nc" not in _CACHE:
        _CACHE["nc"] = _build_nc()
    nc = _CACHE["nc"]

    x = np.ascontiguousarray(np.asarray(x, dtype=np.float32))
    context = np.ascontiguousarray(np.asarray(context, dtype=np.float32))
    mask_u8 = np.ascontiguousarray(np.asarray(mask).astype(np.uint8))
    shared = {
        "Wq": np.ascontiguousarray(np.asarray(Wq, dtype=np.float32)),
        "Wk": np.ascontiguousarray(np.asarray(Wk, dtype=np.float32)),
        "Wv": np.ascontiguousarray(np.asarray(Wv, dtype=np.float32)),
        "Wo": np.ascontiguousarray(np.asarray(Wo, dtype=np.float32)),
        "bo": np.ascontiguousarray(np.asarray(bo, dtype=np.float32)),
    }
    in_maps = [
        {"x": x[b], "context": context[b], "mask": mask_u8[b], **shared}
        for b in range(_B)
    ]
    res = run_bass_kernel_spmd(nc, in_maps, core_ids=list(range(_B)))
    return np.stack([res.results[b]["out"] for b in range(_B)], axis=0)
